# revision 40
# baseline (speedup 1.0000x reference)
"""CrossAttention Trainium2 kernel (8 NeuronCores), v2.

Sharding: 8 cores = 4 batches x 2 head-groups (4 heads of 64 dims each).
Core c handles batch c//2 and inner-dim slice [g*256:(g+1)*256], g = c%2.
Each core computes a partial output [2048, 1024]; the host sums the two
partials per batch and adds bout.

v2 structure (vs v1): LN stats come from bn_stats on natural-layout tiles
(DVE) instead of PE ones-matmuls + ScalarE squares; the attention AV matmul
is flipped (lhsT=exp[keys,128q], rhs=v[keys,65]) so its PE cost is charged
on N=65 instead of N=512 and the softmax denominator lands as a
per-partition scalar; u is rebuilt into pair-stacked uT tiles with cheap PE
transposes; PSUM->SBUF copies for u/o go to the otherwise-idle Pool engine.

Device pipeline per core:
  bn_stats/bn_aggr (DVE) on natural x/c tiles -> mean/rstd [128, 16]
  -> PE-transpose stats to row layout; rstd rows broadcast (Pool) to [128, n]
  -> kT = Wk.T @ cT (+block-diag rank-1 mean fix), scaled by rstd rows
  -> v  = cT.T @ Wv (+rank-1), scaled per-partition rstd; ones column
  -> per (q-chunk, head-pair): qT tile proj, simT = kT_h.T @ qT_h,
     Exp on ScalarE (scale=1/8), u[q,65] = exp_chunk.T-style flipped matmul,
     normalize by denominator column during PSUM->SBUF copy,
     PE-transpose into pair-stacked uT
  -> o = uT.T @ Wout per q-chunk -> fp32 partial out.
"""

import numpy as np
import ml_dtypes

BF16 = ml_dtypes.bfloat16

B = 4
NSEQ = 2048
D = 1024
HEADS = 8
DH = 64
INNER = HEADS * DH  # 512
GI = INNER // 2  # 256 inner dims per core (4 heads)
GH = 4  # heads per core
EPS = 1e-5
SCALE = DH ** -0.5

P = 128
ST = NSEQ // P  # 16 seq tiles
FT = D // P  # 8 feature tiles
MT = GI // P  # 2 inner tiles (head pairs)
QW = 512  # q chunk width
QC = NSEQ // QW  # 4 q chunks
KT = NSEQ // P  # 16 krow tiles

_CACHE = {}


def _build_nc(with_bias=True):
    import concourse.mybir as mybir
    import concourse.tile as tile
    from concourse import bacc, masks

    f32 = mybir.dt.float32
    bf16 = mybir.dt.bfloat16
    Alu = mybir.AluOpType
    Act = mybir.ActivationFunctionType

    nc = bacc.Bacc(None, target_bir_lowering=False)

    xt = nc.dram_tensor("xt", [D, NSEQ], bf16, kind="ExternalInput")
    ct = nc.dram_tensor("ct", [D, NSEQ], bf16, kind="ExternalInput")
    xn = nc.dram_tensor("xn", [NSEQ, D], bf16, kind="ExternalInput")
    cn = nc.dram_tensor("cn", [NSEQ, D], bf16, kind="ExternalInput")
    ncq = nc.dram_tensor("ncq", [1, GI], bf16, kind="ExternalInput")
    nck = nc.dram_tensor("nck", [1, GI], bf16, kind="ExternalInput")
    ncv = nc.dram_tensor("ncv", [1, GI], bf16, kind="ExternalInput")
    wq = nc.dram_tensor("wq", [D, GI], bf16, kind="ExternalInput")
    wk = nc.dram_tensor("wk", [D, GI], bf16, kind="ExternalInput")
    wv = nc.dram_tensor("wv", [D, GI], bf16, kind="ExternalInput")
    wo = nc.dram_tensor("wo", [GI, D], bf16, kind="ExternalInput")
    bq = nc.dram_tensor("bq", [P, MT], f32, kind="ExternalInput")
    bk = nc.dram_tensor("bk", [P, MT], f32, kind="ExternalInput")
    bv = nc.dram_tensor("bv", [P, GI], f32, kind="ExternalInput")
    o = nc.dram_tensor("o", [NSEQ, D], f32, kind="ExternalOutput")

    with tile.TileContext(nc) as tc:
        with (
            tc.tile_pool(name="const", bufs=1) as const,
            tc.tile_pool(name="persist", bufs=1) as persist,
            tc.tile_pool(name="work", bufs=2) as work,
            tc.tile_pool(name="qtp", bufs=3) as qtp,
            tc.tile_pool(name="stats", bufs=4) as stats,
            tc.tile_pool(name="statp", bufs=1) as statp,
            tc.tile_pool(name="small", bufs=4) as small,
            tc.tile_pool(name="unat", bufs=8) as unat,
            tc.tile_pool(name="outp", bufs=4) as outp,
            tc.tile_pool(name="ps_mm", bufs=2, space="PSUM") as ps_mm,
            tc.tile_pool(name="ps_sim", bufs=2, space="PSUM") as ps_sim,
            tc.tile_pool(name="ps_av", bufs=2, space="PSUM") as ps_av,
        ):
            # ---- constants / weights in SBUF ----
            # weight DMAs split per-kt chunk, in consumption order (k, v, q, o)
            wq_sb = const.tile([P, FT, GI], bf16)
            wk_sb = const.tile([P, FT, GI], bf16)
            wv_sb = const.tile([P, FT, GI], bf16)
            wo_sb = const.tile([P, MT, D], bf16)
            wq_r = wq.rearrange("(ko p) m -> p ko m", p=P)
            wk_r = wk.rearrange("(ko p) m -> p ko m", p=P)
            wv_r = wv.rearrange("(ko p) m -> p ko m", p=P)
            for kt in range(FT):
                nc.sync.dma_start(wk_sb[:, kt, :], wk_r[:, kt, :])
            for kt in range(FT):
                nc.sync.dma_start(wv_sb[:, kt, :], wv_r[:, kt, :])
            for kt in range(FT):
                nc.sync.dma_start(wq_sb[:, kt, :], wq_r[:, kt, :])
            for mt in range(MT):
                nc.sync.dma_start(
                    wo_sb[:, mt, :], wo.rearrange("(mt p) d -> p mt d", p=P)[:, mt, :]
                )
            ncq_sb = const.tile([1, GI], bf16)
            nc.sync.dma_start(ncq_sb, ncq[:, :])
            nck_sb = const.tile([1, GI], bf16)
            nc.sync.dma_start(nck_sb, nck[:, :])
            ncv_sb = const.tile([1, GI], bf16)
            nc.sync.dma_start(ncv_sb, ncv[:, :])
            bq_sb = const.tile([P, MT], f32)
            nc.sync.dma_start(bq_sb, bq[:, :])
            bk_sb = const.tile([P, MT], f32)
            nc.sync.dma_start(bk_sb, bk[:, :])
            bv_sb = const.tile([P, GI], f32)
            nc.sync.dma_start(bv_sb, bv[:, :])
            eps_sb = const.tile([P, 1], f32)
            nc.vector.memset(eps_sb, EPS)
            ident_f32 = const.tile([P, P], f32)
            masks.make_identity(nc, ident_f32[:])

            # ---- persistent activations ----
            ct_pool = tc.alloc_tile_pool(name="ctp", bufs=1)
            cT_sb = ct_pool.tile([P, FT, NSEQ], bf16)
            natp = tc.alloc_tile_pool(name="natp", bufs=4)
            xT_sb = persist.tile([P, FT, NSEQ], bf16)
            kT = persist.tile([P, MT, NSEQ], bf16)
            vext = persist.tile([P, KT, GH, DH + 1], bf16)
            uTp = [
                persist.tile([P, NSEQ], bf16, name=f"uTp{m}", tag=f"uTp{m}")
                for m in range(MT)
            ]
            rsb = [
                persist.tile([P, NSEQ], bf16, name=f"rsb{i}", tag=f"rsb{i}")
                for i in range(2)
            ]
            # stats accumulators: mv[ti][:, st, 0]=mean, [..,1]=var
            mv = [
                statp.tile([P, ST, 2], f32, name=f"mv{i}", tag=f"mv{i}")
                for i in range(2)
            ]
            rs_nat = [
                statp.tile([P, ST], f32, name=f"rsn{i}", tag=f"rsn{i}")
                for i in range(2)
            ]
            # row-layout stats at partition 0 (for rank-1 mean fixes / pb)
            mu_bf = [
                statp.tile([1, NSEQ], bf16, name=f"mubf{i}", tag=f"mubf{i}")
                for i in range(2)
            ]
            rs_row = [
                statp.tile([1, NSEQ], bf16, name=f"rsrow{i}", tag=f"rsrow{i}")
                for i in range(2)
            ]

            nc.vector.memset(vext[:, :, :, DH], 1.0)

            # ---- Phase 1: natural-tile stats (DVE) + transposed loads ----
            xt_r = xt.rearrange("(ko p) s -> p ko s", p=P)
            ct_r = ct.rearrange("(ko p) s -> p ko s", p=P)

            def nat_stats(src, ti, st):
                t = natp.tile([P, D], bf16, tag="nat")
                nc.sync.dma_start(t, src[st * P : (st + 1) * P, :])
                bst = stats.tile([P, 2, 6], f32, tag="bst")
                for c2 in range(2):
                    nc.vector.bn_stats(out=bst[:, c2, :], in_=t[:, c2 * 512 : (c2 + 1) * 512])
                nc.vector.bn_aggr(out=mv[ti][:, st, :], in_=bst)

            for st in range(ST):
                nat_stats(cn, 1, st)
                if st < FT:
                    nc.sync.dma_start(cT_sb[:, st, :], ct_r[:, st, :])
                else:
                    nc.sync.dma_start(xT_sb[:, st - FT, :], xt_r[:, st - FT, :])
                nat_stats(xn, 0, st)
            natp.release()

            def stat_rows(ti):
                # rstd per seq position (natural layout)
                sd = stats.tile([P, ST], f32, tag="sd")
                nc.scalar.activation(
                    out=sd, in_=mv[ti][:, :, 1], func=Act.Sqrt, bias=eps_sb
                )
                nc.vector.reciprocal(out=rs_nat[ti], in_=sd)
                # transpose mean and rstd to row layout via PE
                pmu = ps_mm.tile([P, QW], f32, tag="mm")
                nc.tensor.transpose(pmu[0:ST, 0:P], mv[ti][:, :, 0], ident_f32[:])
                nc.tensor.transpose(pmu[0:ST, P : 2 * P], rs_nat[ti][:, :], ident_f32[:])
                rsT = stats.tile([ST, P], bf16, tag="rsT")
                nc.vector.tensor_copy(out=rsT, in_=pmu[0:ST, P : 2 * P])
                muT = stats.tile([ST, P], bf16, tag="muT")
                nc.vector.tensor_copy(out=muT, in_=pmu[0:ST, 0:P])
                for st in range(ST):
                    nc.sync.dma_start(
                        mu_bf[ti][0:1, st * P : (st + 1) * P], muT[st : st + 1, :]
                    )
                    nc.sync.dma_start(
                        rs_row[ti][0:1, st * P : (st + 1) * P], rsT[st : st + 1, :]
                    )
                nc.gpsimd.partition_broadcast(rsb[ti], rs_row[ti])

            stat_rows(1)
            stat_rows(0)

            def proj_qk(w_sb, b_sb, nc_sb, src_T, ti, dst_ap, mt, qc):
                pm = ps_mm.tile([P, QW], f32, tag="mm")
                for kt in range(FT):
                    nc.tensor.matmul(
                        pm,
                        lhsT=w_sb[:, kt, mt * P : (mt + 1) * P],
                        rhs=src_T[:, kt, qc * QW : (qc + 1) * QW],
                        start=(kt == 0),
                        stop=False,
                    )
                # mean fix: rank-1 update (-colsum x mu)
                nc.tensor.matmul(
                    pm,
                    lhsT=nc_sb[0:1, mt * P : (mt + 1) * P],
                    rhs=mu_bf[ti][0:1, qc * QW : (qc + 1) * QW],
                    start=False,
                    stop=True,
                )
                cs = slice(qc * QW, (qc + 1) * QW)
                if with_bias:
                    t1 = work.tile([P, QW], f32, tag="projt")
                    nc.vector.tensor_tensor(out=t1, in0=pm, in1=rsb[ti][:, cs], op=Alu.mult)
                    nc.vector.tensor_scalar(
                        out=dst_ap,
                        in0=t1,
                        scalar1=b_sb[:, mt : mt + 1],
                        scalar2=None,
                        op0=Alu.add,
                    )
                else:
                    nc.vector.tensor_tensor(
                        out=dst_ap, in0=pm, in1=rsb[ti][:, cs], op=Alu.mult
                    )

            def emit_kT(mt):
                for qc2 in range(QC):
                    proj_qk(
                        wk_sb, bk_sb, nck_sb, cT_sb, 1,
                        kT[:, mt, qc2 * QW : (qc2 + 1) * QW], mt, qc2,
                    )

            def emit_v(st_lo, st_hi):
                for st in range(st_lo, st_hi):
                    pm = ps_av.tile([P, GI], f32, tag="av", name="pmv")
                    for kt in range(FT):
                        nc.tensor.matmul(
                            pm,
                            lhsT=cT_sb[:, kt, st * P : (st + 1) * P],
                            rhs=wv_sb[:, kt, :],
                            start=(kt == 0),
                            stop=False,
                        )
                    nc.tensor.matmul(
                        pm,
                        lhsT=mu_bf[1][0:1, st * P : (st + 1) * P],
                        rhs=ncv_sb[0:1, :],
                        start=False,
                        stop=True,
                    )
                    if with_bias:
                        t1 = work.tile([P, GI], f32, tag="vt")
                        nc.vector.tensor_scalar(
                            out=t1,
                            in0=pm,
                            scalar1=rs_nat[1][:, st : st + 1],
                            scalar2=None,
                            op0=Alu.mult,
                        )
                        nc.vector.tensor_tensor(
                            out=vext[:, st, :, 0:DH],
                            in0=t1.rearrange("p (h d) -> p h d", h=GH),
                            in1=bv_sb.rearrange("p (h d) -> p h d", h=GH),
                            op=Alu.add,
                        )
                    else:
                        nc.vector.tensor_scalar(
                            out=vext[:, st, :, 0:DH],
                            in0=pm.rearrange("p (h d) -> p h d", h=GH),
                            scalar1=rs_nat[1][:, st : st + 1],
                            scalar2=None,
                            op0=Alu.mult,
                        )

            expp = tc.alloc_tile_pool(name="expp", bufs=3)

            # ---- attention, software-pipelined at (qc, mt, par) units ----
            # PE order per unit j: [qT proj if new block] [sims U_j]
            # [avs U_{j-1}] [transposes/uT/o-proj when a block completes],
            # so PE's av work overlaps ScalarE's exp work on the next unit.
            un2s = {}  # (qs) staging for the current block

            def sims_unit(qc, mt, par):
                exs = [None, None]
                po = par * DH
                qt = qts[(mt, qc)]
                for kt2 in range(KT // 2):
                    half = kt2 // 4
                    if kt2 % 4 == 0:
                        exs[half] = expp.tile(
                            [P, KT // 2, QW],
                            bf16,
                            tag=f"exp{par}",
                            name=f"exp{par}h{half}",
                        )
                    pm = ps_sim.tile([P, 2, QW], f32, tag="sim")
                    for j in range(2):
                        kt = kt2 * 2 + j
                        nc.tensor.matmul(
                            pm[:, j, :],
                            lhsT=kT[po : po + DH, mt, kt * P : (kt + 1) * P],
                            rhs=qt[po : po + DH, :],
                            start=True,
                            stop=True,
                        )
                    nc.scalar.activation(
                        out=exs[half][:, (kt2 % 4) * 2 : (kt2 % 4) * 2 + 2, :],
                        in_=pm,
                        func=Act.Exp,
                        scale=SCALE,
                    )
                return exs

            def avs_unit(qc, mt, par, exs):
                h = 2 * mt + par
                for qs in range(4):
                    if par == 0:
                        un2s[qs] = unat.tile(
                            [P, 2, DH], f32, tag="un", name=f"un{qs}"
                        )
                    pu = ps_av.tile([P, DH + 1], f32, tag="av")
                    for kt in range(KT):
                        nc.tensor.matmul(
                            pu,
                            lhsT=exs[kt // 8][:, kt % 8, qs * P : (qs + 1) * P],
                            rhs=vext[:, kt, h, :],
                            start=(kt == 0),
                            stop=(kt == KT - 1),
                        )
                    rden = small.tile([P, 1], f32, tag="rden")
                    nc.vector.reciprocal(out=rden, in_=pu[:, DH : DH + 1])
                    nc.vector.tensor_scalar(
                        out=un2s[qs][:, par, :],
                        in0=pu[:, 0:DH],
                        scalar1=rden,
                        scalar2=None,
                        op0=Alu.mult,
                    )

            def finish_block(qc, mt):
                # transpose both heads' u into pair-stacked uT
                ut_ps = ps_mm.tile([P, QW], f32, tag="mm")
                for qs in range(4):
                    nc.tensor.transpose(
                        ut_ps[:, qs * P : (qs + 1) * P], un2s[qs][:, :, :],
                        ident_f32[:],
                    )
                nc.vector.tensor_copy(
                    out=uTp[mt][:, qc * QW : (qc + 1) * QW], in_=ut_ps
                )
                if mt != MT - 1:
                    return
                # output projection for this q-chunk
                for st in range(qc * QC, (qc + 1) * QC):
                    for nck2 in range(2):
                        pm = ps_mm.tile([P, QW], f32, tag="mm")
                        for mt2 in range(MT):
                            nc.tensor.matmul(
                                pm,
                                lhsT=uTp[mt2][:, st * P : (st + 1) * P],
                                rhs=wo_sb[:, mt2, nck2 * QW : (nck2 + 1) * QW],
                                start=(mt2 == 0),
                                stop=(mt2 == MT - 1),
                            )
                        o_sb = outp.tile([P, QW], f32, tag="o")
                        nc.vector.tensor_copy(out=o_sb, in_=pm)
                        nc.sync.dma_start(
                            o[st * P : (st + 1) * P, nck2 * QW : (nck2 + 1) * QW], o_sb
                        )

            # mt-major unit order: kT(mt1) is not needed until unit 8, and
            # the v projection hides under the first two units' exps
            units = [
                (qc, mt, par)
                for mt in range(MT)
                for qc in range(QC)
                for par in range(2)
            ]
            qts = {}
            exs_store = {}

            def do_sims(j):
                qc, mt, par = units[j]
                if par == 0:
                    qt = qtp.tile([P, QW], bf16, tag="qt", name=f"qt{mt}_{qc}")
                    qts[(mt, qc)] = qt
                    proj_qk(wq_sb, bq_sb, ncq_sb, xT_sb, 0, qt[:, :], mt, qc)
                exs_store[j] = sims_unit(qc, mt, par)

            def do_avs(j):
                qc, mt, par = units[j]
                avs_unit(qc, mt, par, exs_store.pop(j))
                if par == 1:
                    finish_block(qc, mt)

            emit_kT(0)
            do_sims(0)
            do_sims(1)
            emit_v(0, ST)
            for j in range(2, 16):
                do_avs(j - 2)
                if j == 8:
                    emit_kT(1)
                do_sims(j)
            do_avs(14)
            do_avs(15)

            expp.release()
            ct_pool.release()

    nc.finalize()
    return nc


def _prep_inputs(x, context, g1, b1, g2, b2, Wq, Wkv, Wout):
    """Fold LN affine into weights; build per-core input maps."""
    f32 = np.float32
    Wqf = (g1[:, None] * Wq).astype(f32)
    bqf = (b1 @ Wq).astype(f32)
    Wkvf = (g2[:, None] * Wkv).astype(f32)
    bkvf = (b2 @ Wkv).astype(f32)
    in_maps = []
    for c in range(8):
        b, g = c // 2, c % 2
        sl = slice(g * GI, (g + 1) * GI)
        slv = slice(INNER + g * GI, INNER + (g + 1) * GI)
        bq_g = bqf[sl.start : sl.stop]
        bk_g = bkvf[sl.start : sl.stop]
        bv_g = bkvf[slv.start : slv.stop]
        ncq_h = -Wqf[:, sl].sum(0)[None, :]
        nck_h = -Wkvf[:, sl].sum(0)[None, :]
        ncv_h = -Wkvf[:, slv].sum(0)[None, :]
        in_maps.append(
            {
                "xt": np.ascontiguousarray(x[b].astype(BF16).T),
                "ct": np.ascontiguousarray(context[b].astype(BF16).T),
                "xn": np.ascontiguousarray(x[b]).astype(BF16),
                "cn": np.ascontiguousarray(context[b]).astype(BF16),
                "ncq": np.ascontiguousarray(ncq_h).astype(BF16),
                "nck": np.ascontiguousarray(nck_h).astype(BF16),
                "ncv": np.ascontiguousarray(ncv_h).astype(BF16),
                "wq": np.ascontiguousarray(Wqf[:, sl]).astype(BF16),
                "wk": np.ascontiguousarray(Wkvf[:, sl]).astype(BF16),
                "wv": np.ascontiguousarray(Wkvf[:, slv]).astype(BF16),
                "wo": np.ascontiguousarray(Wout[sl]).astype(BF16),
                "bq": np.ascontiguousarray(bq_g.reshape(MT, P).T).astype(f32),
                "bk": np.ascontiguousarray(bk_g.reshape(MT, P).T).astype(f32),
                "bv": np.ascontiguousarray(np.broadcast_to(bv_g, (P, GI))).astype(f32),
            }
        )
    return in_maps


def kernel(x, context, g1, b1, g2, b2, Wq, Wkv, Wout, bout, _trace=False):
    from concourse.bass_utils import run_bass_kernel_spmd

    with_bias = bool(np.any(np.asarray(b1)) or np.any(np.asarray(b2)))
    key = ("nc", with_bias)
    if key not in _CACHE:
        _CACHE[key] = _build_nc(with_bias=with_bias)
    nc = _CACHE[key]

    in_maps = _prep_inputs(
        np.asarray(x, np.float32),
        np.asarray(context, np.float32),
        np.asarray(g1, np.float32),
        np.asarray(b1, np.float32),
        np.asarray(g2, np.float32),
        np.asarray(b2, np.float32),
        np.asarray(Wq, np.float32),
        np.asarray(Wkv, np.float32),
        np.asarray(Wout, np.float32),
    )
    res = run_bass_kernel_spmd(nc, in_maps, core_ids=list(range(8)), trace=_trace)
    out = np.empty((B, NSEQ, D), np.float32)
    for b in range(B):
        out[b] = res.results[2 * b]["o"] + res.results[2 * b + 1]["o"]
    out += np.asarray(bout, np.float32)
    _CACHE["last_result"] = res
    return out


# revision 44
# speedup vs baseline: 1.1497x; 1.1497x over previous
"""CrossAttention Trainium2 kernel (8 NeuronCores), v2.

Sharding: 8 cores = 4 batches x 2 head-groups (4 heads of 64 dims each).
Core c handles batch c//2 and inner-dim slice [g*256:(g+1)*256], g = c%2.
Each core computes a partial output [2048, 1024]; the host sums the two
partials per batch and adds bout.

v2 structure (vs v1): LN stats come from bn_stats on natural-layout tiles
(DVE) instead of PE ones-matmuls + ScalarE squares; the attention AV matmul
is flipped (lhsT=exp[keys,128q], rhs=v[keys,65]) so its PE cost is charged
on N=65 instead of N=512 and the softmax denominator lands as a
per-partition scalar; u is rebuilt into pair-stacked uT tiles with cheap PE
transposes; PSUM->SBUF copies for u/o go to the otherwise-idle Pool engine.

Device pipeline per core:
  bn_stats/bn_aggr (DVE) on natural x/c tiles -> mean/rstd [128, 16]
  -> PE-transpose stats to row layout; rstd rows broadcast (Pool) to [128, n]
  -> kT = Wk.T @ cT (+block-diag rank-1 mean fix), scaled by rstd rows
  -> v  = cT.T @ Wv (+rank-1), scaled per-partition rstd; ones column
  -> per (q-chunk, head-pair): qT tile proj, simT = kT_h.T @ qT_h,
     Exp on ScalarE (scale=1/8), u[q,65] = exp_chunk.T-style flipped matmul,
     normalize by denominator column during PSUM->SBUF copy,
     PE-transpose into pair-stacked uT
  -> o = uT.T @ Wout per q-chunk -> fp32 partial out.
"""

import numpy as np
import ml_dtypes

BF16 = ml_dtypes.bfloat16

B = 4
NSEQ = 2048
D = 1024
HEADS = 8
DH = 64
INNER = HEADS * DH  # 512
GI = INNER // 2  # 256 inner dims per core (4 heads)
GH = 4  # heads per core
EPS = 1e-5
SCALE = DH ** -0.5

P = 128
ST = NSEQ // P  # 16 seq tiles
FT = D // P  # 8 feature tiles
MT = GI // P  # 2 inner tiles (head pairs)
QW = 512  # q chunk width
QC = NSEQ // QW  # 4 q chunks
KT = NSEQ // P  # 16 krow tiles

_CACHE = {}


def _build_nc(with_bias=True):
    import concourse.mybir as mybir
    import concourse.tile as tile
    from concourse import bacc, masks

    f32 = mybir.dt.float32
    bf16 = mybir.dt.bfloat16
    Alu = mybir.AluOpType
    Act = mybir.ActivationFunctionType

    nc = bacc.Bacc(None, target_bir_lowering=False)

    xt = nc.dram_tensor("xt", [D, NSEQ], bf16, kind="ExternalInput")
    ct = nc.dram_tensor("ct", [D, NSEQ], bf16, kind="ExternalInput")
    xn = nc.dram_tensor("xn", [NSEQ, D], bf16, kind="ExternalInput")
    cn = nc.dram_tensor("cn", [NSEQ, D], bf16, kind="ExternalInput")
    ncq = nc.dram_tensor("ncq", [1, GI], bf16, kind="ExternalInput")
    nck = nc.dram_tensor("nck", [1, GI], bf16, kind="ExternalInput")
    ncv = nc.dram_tensor("ncv", [1, GI], bf16, kind="ExternalInput")
    wq = nc.dram_tensor("wq", [D, GI], bf16, kind="ExternalInput")
    wk = nc.dram_tensor("wk", [D, GI], bf16, kind="ExternalInput")
    wv = nc.dram_tensor("wv", [D, GI], bf16, kind="ExternalInput")
    wo = nc.dram_tensor("wo", [GI, D], bf16, kind="ExternalInput")
    bq = nc.dram_tensor("bq", [P, MT], f32, kind="ExternalInput")
    bk = nc.dram_tensor("bk", [P, MT], f32, kind="ExternalInput")
    bv = nc.dram_tensor("bv", [P, GI], f32, kind="ExternalInput")
    o = nc.dram_tensor("o", [NSEQ, D], f32, kind="ExternalOutput")

    with tile.TileContext(nc) as tc:
        with (
            tc.tile_pool(name="const", bufs=1) as const,
            tc.tile_pool(name="persist", bufs=1) as persist,
            tc.tile_pool(name="work", bufs=2) as work,
            tc.tile_pool(name="qtp", bufs=3) as qtp,
            tc.tile_pool(name="stats", bufs=4) as stats,
            tc.tile_pool(name="statp", bufs=1) as statp,
            tc.tile_pool(name="small", bufs=4) as small,
            tc.tile_pool(name="unat", bufs=8) as unat,
            tc.tile_pool(name="outp", bufs=4) as outp,
            tc.tile_pool(name="ps_mm", bufs=2, space="PSUM") as ps_mm,
            tc.tile_pool(name="ps_sim", bufs=2, space="PSUM") as ps_sim,
            tc.tile_pool(name="ps_av", bufs=2, space="PSUM") as ps_av,
        ):
            # ---- constants / weights in SBUF ----
            # weight tiles; DMAs are issued later in consumption order
            wq_sb = const.tile([P, FT, GI], bf16)
            wk_sb = const.tile([P, FT, GI], bf16)
            wv_sb = const.tile([P, FT, GI], bf16)
            wo_sb = const.tile([P, MT, D], bf16)
            ncq_sb = const.tile([1, GI], bf16)
            nc.sync.dma_start(ncq_sb, ncq[:, :])
            nck_sb = const.tile([1, GI], bf16)
            nc.sync.dma_start(nck_sb, nck[:, :])
            ncv_sb = const.tile([1, GI], bf16)
            nc.sync.dma_start(ncv_sb, ncv[:, :])
            bq_sb = const.tile([P, MT], f32)
            nc.sync.dma_start(bq_sb, bq[:, :])
            bk_sb = const.tile([P, MT], f32)
            nc.sync.dma_start(bk_sb, bk[:, :])
            bv_sb = const.tile([P, GI], f32)
            nc.sync.dma_start(bv_sb, bv[:, :])
            eps_sb = const.tile([P, 1], f32)
            nc.vector.memset(eps_sb, EPS)
            ident_f32 = const.tile([P, P], f32)
            masks.make_identity(nc, ident_f32[:])

            # ---- persistent activations ----
            ct_pool = tc.alloc_tile_pool(name="ctp", bufs=1)
            cT_sb = ct_pool.tile([P, FT, NSEQ], bf16)
            natp = tc.alloc_tile_pool(name="natp", bufs=4)
            xT_sb = persist.tile([P, FT, NSEQ], bf16)
            kT = persist.tile([P, MT, NSEQ], bf16)
            vext = persist.tile([P, KT, GH, DH + 1], bf16)
            uTp = [
                persist.tile([P, NSEQ], bf16, name=f"uTp{m}", tag=f"uTp{m}")
                for m in range(MT)
            ]
            rsb = [
                persist.tile([P, NSEQ], bf16, name=f"rsb{i}", tag=f"rsb{i}")
                for i in range(2)
            ]
            # stats accumulators: mv[ti][:, st, 0]=mean, [..,1]=var
            mv = [
                statp.tile([P, ST, 2], f32, name=f"mv{i}", tag=f"mv{i}")
                for i in range(2)
            ]
            rs_nat = [
                statp.tile([P, ST], f32, name=f"rsn{i}", tag=f"rsn{i}")
                for i in range(2)
            ]
            # row-layout stats at partition 0 (for rank-1 mean fixes / pb)
            mu_bf = [
                statp.tile([1, NSEQ], bf16, name=f"mubf{i}", tag=f"mubf{i}")
                for i in range(2)
            ]
            rs_row = [
                statp.tile([1, NSEQ], bf16, name=f"rsrow{i}", tag=f"rsrow{i}")
                for i in range(2)
            ]

            nc.vector.memset(vext[:, :, :, DH], 1.0)

            # x sums for the ScalarE-computed half of x stats
            sx_sum = statp.tile([P, ST], f32, name="sxsum", tag="sxsum")
            sx_sq = statp.tile([P, ST], f32, name="sxsq", tag="sxsq")

            # ---- Phase 1: stats + loads; DMAs batched and issued in
            # consumption order (c-nat, cT, wk -> x-nat, xT blocks, wq, wv) --
            xt_r = xt.rearrange("(ko p) s -> p ko s", p=P)
            ct_r = ct.rearrange("(ko p) s -> p ko s", p=P)
            wq_r = wq.rearrange("(ko p) m -> p ko m", p=P)
            wk_r = wk.rearrange("(ko p) m -> p ko m", p=P)
            wv_r = wv.rearrange("(ko p) m -> p ko m", p=P)
            wo_r = wo.rearrange("(mt p) d -> p mt d", p=P)

            BT = 4  # seq tiles per big natural tile
            nat_tiles = {}

            def nat_load(src, ti, bt):
                t = natp.tile([P, BT, D], bf16, tag="nat", name=f"nat{ti}_{bt}")
                nc.sync.dma_start(
                    t, src[bt * BT * P : (bt + 1) * BT * P, :].rearrange(
                        "(s p) d -> p s d", p=P
                    )
                )
                nat_tiles[(ti, bt)] = t

            def dve_stats(ti, bt, s):
                t = nat_tiles[(ti, bt)]
                st = bt * BT + s
                bst = stats.tile([P, 2, 6], f32, tag="bst")
                for c2 in range(2):
                    nc.vector.bn_stats(
                        out=bst[:, c2, :], in_=t[:, s, c2 * 512 : (c2 + 1) * 512]
                    )
                nc.vector.bn_aggr(out=mv[ti][:, st, :], in_=bst)

            def se_stats(ti, bt, s):
                # x stats on ScalarE: Square+accum and Copy+accum
                t = nat_tiles[(ti, bt)]
                st = bt * BT + s
                scr = stats.tile([P, D], bf16, tag="sescr")
                nc.scalar.activation(
                    out=scr, in_=t[:, s, :], func=Act.Square,
                    accum_out=sx_sq[:, st : st + 1],
                )
                scr2 = stats.tile([P, D], bf16, tag="sescr")
                nc.scalar.activation(
                    out=scr2, in_=t[:, s, :], func=Act.Copy,
                    accum_out=sx_sum[:, st : st + 1],
                )

            # c natural + cT + wk first (kT path), then x
            for bt in range(ST // BT):
                nat_load(cn, 1, bt)
            for qc2 in range(QC):
                nc.sync.dma_start(
                    cT_sb[:, :, qc2 * QW : (qc2 + 1) * QW],
                    ct_r[:, :, qc2 * QW : (qc2 + 1) * QW],
                )
            nc.sync.dma_start(wk_sb, wk_r)
            for bt in range(ST // BT):
                for s in range(BT):
                    dve_stats(1, bt, s)
                nat_load(xn, 0, bt)
            nc.sync.dma_start(wv_sb, wv_r)
            nc.sync.dma_start(wq_sb, wq_r)
            for qc2 in range(QC):
                nc.sync.dma_start(
                    xT_sb[:, :, qc2 * QW : (qc2 + 1) * QW],
                    xt_r[:, :, qc2 * QW : (qc2 + 1) * QW],
                )
            nc.sync.dma_start(wo_sb, wo_r)
            # x stats: first half DVE, second half ScalarE
            for bt in range(ST // BT):
                for s in range(BT):
                    if bt < 2:
                        dve_stats(0, bt, s)
                    else:
                        se_stats(0, bt, s)
            # finalize ScalarE-half x stats: mean=sum/D, var=sumsq/D-mean^2
            hs = slice(ST // 2, ST)
            nc.vector.tensor_scalar(
                out=mv[0][:, hs, 0], in0=sx_sum[:, hs], scalar1=1.0 / D,
                scalar2=None, op0=Alu.mult,
            )
            m2 = stats.tile([P, ST // 2], f32, tag="m2")
            nc.vector.tensor_tensor(
                out=m2, in0=mv[0][:, hs, 0], in1=mv[0][:, hs, 0], op=Alu.mult
            )
            nc.vector.tensor_scalar(
                out=mv[0][:, hs, 1], in0=sx_sq[:, hs], scalar1=1.0 / D,
                scalar2=None, op0=Alu.mult,
            )
            nc.vector.tensor_tensor(
                out=mv[0][:, hs, 1], in0=mv[0][:, hs, 1], in1=m2, op=Alu.subtract
            )
            natp.release()

            def stat_rows(ti):
                # rstd per seq position (natural layout)
                sd = stats.tile([P, ST], f32, tag="sd")
                nc.scalar.activation(
                    out=sd, in_=mv[ti][:, :, 1], func=Act.Sqrt, bias=eps_sb
                )
                nc.vector.reciprocal(out=rs_nat[ti], in_=sd)
                # transpose mean and rstd to row layout via PE
                pmu = ps_mm.tile([P, QW], f32, tag="mm")
                nc.tensor.transpose(pmu[0:ST, 0:P], mv[ti][:, :, 0], ident_f32[:])
                nc.tensor.transpose(pmu[0:ST, P : 2 * P], rs_nat[ti][:, :], ident_f32[:])
                rsT = stats.tile([ST, P], bf16, tag="rsT")
                nc.vector.tensor_copy(out=rsT, in_=pmu[0:ST, P : 2 * P])
                muT = stats.tile([ST, P], bf16, tag="muT")
                nc.vector.tensor_copy(out=muT, in_=pmu[0:ST, 0:P])
                nc.sync.dma_start(mu_bf[ti][0:1, :], muT[:, :])
                nc.sync.dma_start(rs_row[ti][0:1, :], rsT[:, :])
                nc.gpsimd.partition_broadcast(rsb[ti], rs_row[ti])

            stat_rows(1)
            stat_rows(0)

            def proj_qk(w_sb, b_sb, nc_sb, src_T, ti, dst_ap, mt, qc):
                pm = ps_mm.tile([P, QW], f32, tag="mm")
                for kt in range(FT):
                    nc.tensor.matmul(
                        pm,
                        lhsT=w_sb[:, kt, mt * P : (mt + 1) * P],
                        rhs=src_T[:, kt, qc * QW : (qc + 1) * QW],
                        start=(kt == 0),
                        stop=False,
                    )
                # mean fix: rank-1 update (-colsum x mu)
                nc.tensor.matmul(
                    pm,
                    lhsT=nc_sb[0:1, mt * P : (mt + 1) * P],
                    rhs=mu_bf[ti][0:1, qc * QW : (qc + 1) * QW],
                    start=False,
                    stop=True,
                )
                cs = slice(qc * QW, (qc + 1) * QW)
                if with_bias:
                    t1 = work.tile([P, QW], f32, tag="projt")
                    nc.vector.tensor_tensor(out=t1, in0=pm, in1=rsb[ti][:, cs], op=Alu.mult)
                    nc.vector.tensor_scalar(
                        out=dst_ap,
                        in0=t1,
                        scalar1=b_sb[:, mt : mt + 1],
                        scalar2=None,
                        op0=Alu.add,
                    )
                else:
                    nc.vector.tensor_tensor(
                        out=dst_ap, in0=pm, in1=rsb[ti][:, cs], op=Alu.mult
                    )

            def emit_kT(mt):
                for qc2 in range(QC):
                    proj_qk(
                        wk_sb, bk_sb, nck_sb, cT_sb, 1,
                        kT[:, mt, qc2 * QW : (qc2 + 1) * QW], mt, qc2,
                    )

            def emit_v(st_lo, st_hi):
                for st in range(st_lo, st_hi):
                    pm = ps_av.tile([P, GI], f32, tag="av", name="pmv")
                    for kt in range(FT):
                        nc.tensor.matmul(
                            pm,
                            lhsT=cT_sb[:, kt, st * P : (st + 1) * P],
                            rhs=wv_sb[:, kt, :],
                            start=(kt == 0),
                            stop=False,
                        )
                    nc.tensor.matmul(
                        pm,
                        lhsT=mu_bf[1][0:1, st * P : (st + 1) * P],
                        rhs=ncv_sb[0:1, :],
                        start=False,
                        stop=True,
                    )
                    if with_bias:
                        t1 = work.tile([P, GI], f32, tag="vt")
                        nc.vector.tensor_scalar(
                            out=t1,
                            in0=pm,
                            scalar1=rs_nat[1][:, st : st + 1],
                            scalar2=None,
                            op0=Alu.mult,
                        )
                        nc.vector.tensor_tensor(
                            out=vext[:, st, :, 0:DH],
                            in0=t1.rearrange("p (h d) -> p h d", h=GH),
                            in1=bv_sb.rearrange("p (h d) -> p h d", h=GH),
                            op=Alu.add,
                        )
                    else:
                        nc.vector.tensor_scalar(
                            out=vext[:, st, :, 0:DH],
                            in0=pm.rearrange("p (h d) -> p h d", h=GH),
                            scalar1=rs_nat[1][:, st : st + 1],
                            scalar2=None,
                            op0=Alu.mult,
                        )

            expp = tc.alloc_tile_pool(name="expp", bufs=3)

            # ---- attention, software-pipelined at (qc, mt, par) units ----
            # PE order per unit j: [qT proj if new block] [sims U_j]
            # [avs U_{j-1}] [transposes/uT/o-proj when a block completes],
            # so PE's av work overlaps ScalarE's exp work on the next unit.
            un2s = {}  # (qs) staging for the current block

            def sims_unit(qc, mt, par):
                exs = [None, None]
                po = par * DH
                qt = qts[(mt, qc)]
                for kt2 in range(KT // 2):
                    half = kt2 // 4
                    if kt2 % 4 == 0:
                        exs[half] = expp.tile(
                            [P, KT // 2, QW],
                            bf16,
                            tag=f"exp{par}",
                            name=f"exp{par}h{half}",
                        )
                    pm = ps_sim.tile([P, 2, QW], f32, tag="sim")
                    for j in range(2):
                        kt = kt2 * 2 + j
                        nc.tensor.matmul(
                            pm[:, j, :],
                            lhsT=kT[po : po + DH, mt, kt * P : (kt + 1) * P],
                            rhs=qt[po : po + DH, :],
                            start=True,
                            stop=True,
                        )
                    nc.scalar.activation(
                        out=exs[half][:, (kt2 % 4) * 2 : (kt2 % 4) * 2 + 2, :],
                        in_=pm,
                        func=Act.Exp,
                        scale=SCALE,
                    )
                return exs

            def avs_unit(qc, mt, par, exs):
                h = 2 * mt + par
                for qs in range(4):
                    if par == 0:
                        un2s[qs] = unat.tile(
                            [P, 2, DH], f32, tag="un", name=f"un{qs}"
                        )
                    pu = ps_av.tile([P, DH + 1], f32, tag="av")
                    for kt in range(KT):
                        nc.tensor.matmul(
                            pu,
                            lhsT=exs[kt // 8][:, kt % 8, qs * P : (qs + 1) * P],
                            rhs=vext[:, kt, h, :],
                            start=(kt == 0),
                            stop=(kt == KT - 1),
                        )
                    rden = small.tile([P, 1], f32, tag="rden")
                    nc.vector.reciprocal(out=rden, in_=pu[:, DH : DH + 1])
                    nc.vector.tensor_scalar(
                        out=un2s[qs][:, par, :],
                        in0=pu[:, 0:DH],
                        scalar1=rden,
                        scalar2=None,
                        op0=Alu.mult,
                    )

            def finish_block(qc, mt):
                # transpose both heads' u into pair-stacked uT
                ut_ps = ps_mm.tile([P, QW], f32, tag="mm")
                for qs in range(4):
                    nc.tensor.transpose(
                        ut_ps[:, qs * P : (qs + 1) * P], un2s[qs][:, :, :],
                        ident_f32[:],
                    )
                nc.vector.tensor_copy(
                    out=uTp[mt][:, qc * QW : (qc + 1) * QW], in_=ut_ps
                )
                if mt != MT - 1:
                    return
                # output projection for this q-chunk
                for st in range(qc * QC, (qc + 1) * QC):
                    for nck2 in range(2):
                        pm = ps_mm.tile([P, QW], f32, tag="mm")
                        for mt2 in range(MT):
                            nc.tensor.matmul(
                                pm,
                                lhsT=uTp[mt2][:, st * P : (st + 1) * P],
                                rhs=wo_sb[:, mt2, nck2 * QW : (nck2 + 1) * QW],
                                start=(mt2 == 0),
                                stop=(mt2 == MT - 1),
                            )
                        o_sb = outp.tile([P, QW], f32, tag="o")
                        nc.vector.tensor_copy(out=o_sb, in_=pm)
                        nc.sync.dma_start(
                            o[st * P : (st + 1) * P, nck2 * QW : (nck2 + 1) * QW], o_sb
                        )

            # mt-major unit order: kT(mt1) is not needed until unit 8, and
            # the v projection hides under the first two units' exps
            units = [
                (qc, mt, par)
                for mt in range(MT)
                for qc in range(QC)
                for par in range(2)
            ]
            qts = {}
            exs_store = {}

            def do_sims(j):
                qc, mt, par = units[j]
                if par == 0:
                    qt = qtp.tile([P, QW], bf16, tag="qt", name=f"qt{mt}_{qc}")
                    qts[(mt, qc)] = qt
                    proj_qk(wq_sb, bq_sb, ncq_sb, xT_sb, 0, qt[:, :], mt, qc)
                exs_store[j] = sims_unit(qc, mt, par)

            def do_avs(j):
                qc, mt, par = units[j]
                avs_unit(qc, mt, par, exs_store.pop(j))
                if par == 1:
                    finish_block(qc, mt)

            emit_kT(0)
            do_sims(0)
            do_sims(1)
            emit_v(0, ST)
            for j in range(2, 16):
                do_avs(j - 2)
                if j == 8:
                    emit_kT(1)
                do_sims(j)
            do_avs(14)
            do_avs(15)

            expp.release()
            ct_pool.release()

    nc.finalize()
    return nc


def _prep_inputs(x, context, g1, b1, g2, b2, Wq, Wkv, Wout):
    """Fold LN affine into weights; build per-core input maps."""
    f32 = np.float32
    Wqf = (g1[:, None] * Wq).astype(f32)
    bqf = (b1 @ Wq).astype(f32)
    Wkvf = (g2[:, None] * Wkv).astype(f32)
    bkvf = (b2 @ Wkv).astype(f32)
    in_maps = []
    for c in range(8):
        b, g = c // 2, c % 2
        sl = slice(g * GI, (g + 1) * GI)
        slv = slice(INNER + g * GI, INNER + (g + 1) * GI)
        bq_g = bqf[sl.start : sl.stop]
        bk_g = bkvf[sl.start : sl.stop]
        bv_g = bkvf[slv.start : slv.stop]
        ncq_h = -Wqf[:, sl].sum(0)[None, :]
        nck_h = -Wkvf[:, sl].sum(0)[None, :]
        ncv_h = -Wkvf[:, slv].sum(0)[None, :]
        in_maps.append(
            {
                "xt": np.ascontiguousarray(x[b].astype(BF16).T),
                "ct": np.ascontiguousarray(context[b].astype(BF16).T),
                "xn": np.ascontiguousarray(x[b]).astype(BF16),
                "cn": np.ascontiguousarray(context[b]).astype(BF16),
                "ncq": np.ascontiguousarray(ncq_h).astype(BF16),
                "nck": np.ascontiguousarray(nck_h).astype(BF16),
                "ncv": np.ascontiguousarray(ncv_h).astype(BF16),
                "wq": np.ascontiguousarray(Wqf[:, sl]).astype(BF16),
                "wk": np.ascontiguousarray(Wkvf[:, sl]).astype(BF16),
                "wv": np.ascontiguousarray(Wkvf[:, slv]).astype(BF16),
                "wo": np.ascontiguousarray(Wout[sl]).astype(BF16),
                "bq": np.ascontiguousarray(bq_g.reshape(MT, P).T).astype(f32),
                "bk": np.ascontiguousarray(bk_g.reshape(MT, P).T).astype(f32),
                "bv": np.ascontiguousarray(np.broadcast_to(bv_g, (P, GI))).astype(f32),
            }
        )
    return in_maps


def kernel(x, context, g1, b1, g2, b2, Wq, Wkv, Wout, bout, _trace=False):
    from concourse.bass_utils import run_bass_kernel_spmd

    with_bias = bool(np.any(np.asarray(b1)) or np.any(np.asarray(b2)))
    key = ("nc", with_bias)
    if key not in _CACHE:
        _CACHE[key] = _build_nc(with_bias=with_bias)
    nc = _CACHE[key]

    in_maps = _prep_inputs(
        np.asarray(x, np.float32),
        np.asarray(context, np.float32),
        np.asarray(g1, np.float32),
        np.asarray(b1, np.float32),
        np.asarray(g2, np.float32),
        np.asarray(b2, np.float32),
        np.asarray(Wq, np.float32),
        np.asarray(Wkv, np.float32),
        np.asarray(Wout, np.float32),
    )
    res = run_bass_kernel_spmd(nc, in_maps, core_ids=list(range(8)), trace=_trace)
    out = np.empty((B, NSEQ, D), np.float32)
    for b in range(B):
        out[b] = res.results[2 * b]["o"] + res.results[2 * b + 1]["o"]
    out += np.asarray(bout, np.float32)
    _CACHE["last_result"] = res
    return out


# revision 46
# speedup vs baseline: 1.2005x; 1.0442x over previous
"""CrossAttention Trainium2 kernel (8 NeuronCores), v2.

Sharding: 8 cores = 4 batches x 2 head-groups (4 heads of 64 dims each).
Core c handles batch c//2 and inner-dim slice [g*256:(g+1)*256], g = c%2.
Each core computes a partial output [2048, 1024]; the host sums the two
partials per batch and adds bout.

v2 structure (vs v1): LN stats come from bn_stats on natural-layout tiles
(DVE) instead of PE ones-matmuls + ScalarE squares; the attention AV matmul
is flipped (lhsT=exp[keys,128q], rhs=v[keys,65]) so its PE cost is charged
on N=65 instead of N=512 and the softmax denominator lands as a
per-partition scalar; u is rebuilt into pair-stacked uT tiles with cheap PE
transposes; PSUM->SBUF copies for u/o go to the otherwise-idle Pool engine.

Device pipeline per core:
  bn_stats/bn_aggr (DVE) on natural x/c tiles -> mean/rstd [128, 16]
  -> PE-transpose stats to row layout; rstd rows broadcast (Pool) to [128, n]
  -> kT = Wk.T @ cT (+block-diag rank-1 mean fix), scaled by rstd rows
  -> v  = cT.T @ Wv (+rank-1), scaled per-partition rstd; ones column
  -> per (q-chunk, head-pair): qT tile proj, simT = kT_h.T @ qT_h,
     Exp on ScalarE (scale=1/8), u[q,65] = exp_chunk.T-style flipped matmul,
     normalize by denominator column during PSUM->SBUF copy,
     PE-transpose into pair-stacked uT
  -> o = uT.T @ Wout per q-chunk -> fp32 partial out.
"""

import numpy as np
import ml_dtypes

BF16 = ml_dtypes.bfloat16

B = 4
NSEQ = 2048
D = 1024
HEADS = 8
DH = 64
INNER = HEADS * DH  # 512
GI = INNER // 2  # 256 inner dims per core (4 heads)
GH = 4  # heads per core
EPS = 1e-5
SCALE = DH ** -0.5

P = 128
ST = NSEQ // P  # 16 seq tiles
FT = D // P  # 8 feature tiles
MT = GI // P  # 2 inner tiles (head pairs)
QW = 512  # q chunk width
QC = NSEQ // QW  # 4 q chunks
KT = NSEQ // P  # 16 krow tiles

_CACHE = {}


def _build_nc(with_bias=True):
    import concourse.mybir as mybir
    import concourse.tile as tile
    from concourse import bacc, masks

    f32 = mybir.dt.float32
    bf16 = mybir.dt.bfloat16
    Alu = mybir.AluOpType
    Act = mybir.ActivationFunctionType

    nc = bacc.Bacc(None, target_bir_lowering=False)

    xt = nc.dram_tensor("xt", [D, NSEQ], bf16, kind="ExternalInput")
    ct = nc.dram_tensor("ct", [D, NSEQ], bf16, kind="ExternalInput")
    xn = nc.dram_tensor("xn", [NSEQ, D], bf16, kind="ExternalInput")
    cn = nc.dram_tensor("cn", [NSEQ, D], bf16, kind="ExternalInput")
    ncq = nc.dram_tensor("ncq", [1, GI], bf16, kind="ExternalInput")
    nck = nc.dram_tensor("nck", [1, GI], bf16, kind="ExternalInput")
    ncv = nc.dram_tensor("ncv", [1, GI], bf16, kind="ExternalInput")
    wq = nc.dram_tensor("wq", [D, GI], bf16, kind="ExternalInput")
    wk = nc.dram_tensor("wk", [D, GI], bf16, kind="ExternalInput")
    wv = nc.dram_tensor("wv", [D, GI], bf16, kind="ExternalInput")
    wo = nc.dram_tensor("wo", [GI, D], bf16, kind="ExternalInput")
    bq = nc.dram_tensor("bq", [P, MT], f32, kind="ExternalInput")
    bk = nc.dram_tensor("bk", [P, MT], f32, kind="ExternalInput")
    bv = nc.dram_tensor("bv", [P, GI], f32, kind="ExternalInput")
    o = nc.dram_tensor("o", [NSEQ, D], f32, kind="ExternalOutput")

    with tile.TileContext(nc) as tc:
        with (
            tc.tile_pool(name="const", bufs=1) as const,
            tc.tile_pool(name="persist", bufs=1) as persist,
            tc.tile_pool(name="work", bufs=2) as work,
            tc.tile_pool(name="qtp", bufs=3) as qtp,
            tc.tile_pool(name="stats", bufs=4) as stats,
            tc.tile_pool(name="statp", bufs=1) as statp,
            tc.tile_pool(name="small", bufs=4) as small,
            tc.tile_pool(name="unat", bufs=8) as unat,
            tc.tile_pool(name="outp", bufs=4) as outp,
            tc.tile_pool(name="ps_mm", bufs=2, space="PSUM") as ps_mm,
            tc.tile_pool(name="ps_sim", bufs=2, space="PSUM") as ps_sim,
            tc.tile_pool(name="ps_av", bufs=2, space="PSUM") as ps_av,
        ):
            # ---- constants / weights in SBUF ----
            # weight tiles; DMAs are issued later in consumption order
            wq_sb = const.tile([P, FT, GI], bf16)
            wk_sb = const.tile([P, FT, GI], bf16)
            wv_sb = const.tile([P, FT, GI], bf16)
            wo_sb = const.tile([P, MT, D], bf16)
            ncq_sb = const.tile([1, GI], bf16)
            nc.sync.dma_start(ncq_sb, ncq[:, :])
            nck_sb = const.tile([1, GI], bf16)
            nc.sync.dma_start(nck_sb, nck[:, :])
            ncv_sb = const.tile([1, GI], bf16)
            nc.sync.dma_start(ncv_sb, ncv[:, :])
            bq_sb = const.tile([P, MT], f32)
            nc.sync.dma_start(bq_sb, bq[:, :])
            bk_sb = const.tile([P, MT], f32)
            nc.sync.dma_start(bk_sb, bk[:, :])
            bv_sb = const.tile([P, GI], f32)
            nc.sync.dma_start(bv_sb, bv[:, :])
            eps_sb = const.tile([P, 1], f32)
            nc.vector.memset(eps_sb, EPS)
            ident_f32 = const.tile([P, P], f32)
            masks.make_identity(nc, ident_f32[:])

            # ---- persistent activations ----
            ct_pool = tc.alloc_tile_pool(name="ctp", bufs=1)
            cT_sb = ct_pool.tile([P, FT, NSEQ], bf16)
            natp = tc.alloc_tile_pool(name="natp", bufs=4)
            xT_sb = persist.tile([P, FT, NSEQ], bf16)
            kT = persist.tile([P, MT, NSEQ], bf16)
            vext = persist.tile([P, KT, GH, DH + 1], bf16)
            uTp = [
                persist.tile([P, NSEQ], bf16, name=f"uTp{m}", tag=f"uTp{m}")
                for m in range(MT)
            ]
            rsb = [
                persist.tile([P, NSEQ], bf16, name=f"rsb{i}", tag=f"rsb{i}")
                for i in range(2)
            ]
            # stats accumulators: mv[ti][:, st, 0]=mean, [..,1]=var
            mv = [
                statp.tile([P, ST, 2], f32, name=f"mv{i}", tag=f"mv{i}")
                for i in range(2)
            ]
            rs_nat = [
                statp.tile([P, ST], f32, name=f"rsn{i}", tag=f"rsn{i}")
                for i in range(2)
            ]
            # row-layout stats at partition 0 (for rank-1 mean fixes / pb)
            mu_bf = [
                statp.tile([1, NSEQ], bf16, name=f"mubf{i}", tag=f"mubf{i}")
                for i in range(2)
            ]
            rs_row = [
                statp.tile([1, NSEQ], bf16, name=f"rsrow{i}", tag=f"rsrow{i}")
                for i in range(2)
            ]

            nc.vector.memset(vext[:, :, :, DH], 1.0)

            # x sums for the ScalarE-computed half of x stats
            sx_sum = statp.tile([P, ST], f32, name="sxsum", tag="sxsum")
            sx_sq = statp.tile([P, ST], f32, name="sxsq", tag="sxsq")

            # ---- Phase 1: stats + loads; DMAs batched and issued in
            # consumption order (c-nat, cT, wk -> x-nat, xT blocks, wq, wv) --
            xt_r = xt.rearrange("(ko p) s -> p ko s", p=P)
            ct_r = ct.rearrange("(ko p) s -> p ko s", p=P)
            wq_r = wq.rearrange("(ko p) m -> p ko m", p=P)
            wk_r = wk.rearrange("(ko p) m -> p ko m", p=P)
            wv_r = wv.rearrange("(ko p) m -> p ko m", p=P)
            wo_r = wo.rearrange("(mt p) d -> p mt d", p=P)

            BT = 4  # seq tiles per big natural tile
            nat_tiles = {}

            def nat_load(src, ti, bt):
                t = natp.tile([P, BT, D], bf16, tag="nat", name=f"nat{ti}_{bt}")
                nc.sync.dma_start(
                    t, src[bt * BT * P : (bt + 1) * BT * P, :].rearrange(
                        "(s p) d -> p s d", p=P
                    )
                )
                nat_tiles[(ti, bt)] = t

            def dve_stats(ti, bt, s):
                t = nat_tiles[(ti, bt)]
                st = bt * BT + s
                bst = stats.tile([P, 2, 6], f32, tag="bst")
                for c2 in range(2):
                    nc.vector.bn_stats(
                        out=bst[:, c2, :], in_=t[:, s, c2 * 512 : (c2 + 1) * 512]
                    )
                nc.vector.bn_aggr(out=mv[ti][:, st, :], in_=bst)

            def se_stats(ti, bt, s):
                # x stats on ScalarE: Square+accum and Copy+accum
                t = nat_tiles[(ti, bt)]
                st = bt * BT + s
                scr = stats.tile([P, D], bf16, tag="sescr")
                nc.scalar.activation(
                    out=scr, in_=t[:, s, :], func=Act.Square,
                    accum_out=sx_sq[:, st : st + 1],
                )
                scr2 = stats.tile([P, D], bf16, tag="sescr")
                nc.scalar.activation(
                    out=scr2, in_=t[:, s, :], func=Act.Copy,
                    accum_out=sx_sum[:, st : st + 1],
                )

            def stat_rows(ti):
                # rstd per seq position (natural layout)
                sd = stats.tile([P, ST], f32, tag="sd")
                nc.scalar.activation(
                    out=sd, in_=mv[ti][:, :, 1], func=Act.Sqrt, bias=eps_sb
                )
                nc.vector.reciprocal(out=rs_nat[ti], in_=sd)
                # transpose mean and rstd to row layout via PE
                pmu = ps_mm.tile([P, QW], f32, tag="mm")
                nc.tensor.transpose(pmu[0:ST, 0:P], mv[ti][:, :, 0], ident_f32[:])
                nc.tensor.transpose(pmu[0:ST, P : 2 * P], rs_nat[ti][:, :], ident_f32[:])
                rsT = stats.tile([ST, P], bf16, tag="rsT")
                nc.vector.tensor_copy(out=rsT, in_=pmu[0:ST, P : 2 * P])
                muT = stats.tile([ST, P], bf16, tag="muT")
                nc.vector.tensor_copy(out=muT, in_=pmu[0:ST, 0:P])
                nc.sync.dma_start(mu_bf[ti][0:1, :], muT[:, :])
                nc.sync.dma_start(rs_row[ti][0:1, :], rsT[:, :])
                nc.gpsimd.partition_broadcast(rsb[ti], rs_row[ti])


            # DMA dispatch order tuned to arrival deadlines:
            # c-nat (stats) -> wk -> x-nat 0/1 (ScalarE stats start early)
            # -> cT blocks (kT rhs) -> x-nat 2/3 -> xT block 0 / wq (first
            # q-chunk) -> wv -> remaining xT -> wo
            def dma_cT(qc2):
                nc.sync.dma_start(
                    cT_sb[:, :, qc2 * QW : (qc2 + 1) * QW],
                    ct_r[:, :, qc2 * QW : (qc2 + 1) * QW],
                )

            def dma_xT(qc2):
                nc.sync.dma_start(
                    xT_sb[:, :, qc2 * QW : (qc2 + 1) * QW],
                    xt_r[:, :, qc2 * QW : (qc2 + 1) * QW],
                )

            for bt in range(ST // BT):
                nat_load(cn, 1, bt)
            nc.sync.dma_start(wk_sb, wk_r)
            # c stats for bt 0,1 then x loads take their ring slots
            for s in range(BT):
                dve_stats(1, 0, s)
            nat_load(xn, 0, 0)
            for s in range(BT):
                se_stats(0, 0, s)
            for s in range(BT):
                dve_stats(1, 1, s)
            nat_load(xn, 0, 1)
            for s in range(BT):
                se_stats(0, 1, s)
            dma_cT(0)
            dma_cT(1)
            for s in range(BT):
                dve_stats(1, 2, s)
            nat_load(xn, 0, 2)
            for s in range(BT):
                dve_stats(1, 3, s)
            stat_rows(1)
            nat_load(xn, 0, 3)
            dma_cT(2)
            dma_cT(3)
            dma_xT(0)
            nc.sync.dma_start(wq_sb, wq_r)
            nc.sync.dma_start(wv_sb, wv_r)
            for qc2 in range(1, QC):
                dma_xT(qc2)
            nc.sync.dma_start(wo_sb, wo_r)
            # x stats for bt 2,3 on DVE
            for bt in range(2, ST // BT):
                for s in range(BT):
                    dve_stats(0, bt, s)
            # finalize ScalarE-half x stats: mean=sum/D, var=sumsq/D-mean^2
            hs = slice(0, ST // 2)
            nc.vector.tensor_scalar(
                out=mv[0][:, hs, 0], in0=sx_sum[:, hs], scalar1=1.0 / D,
                scalar2=None, op0=Alu.mult,
            )
            m2 = stats.tile([P, ST // 2], f32, tag="m2")
            nc.vector.tensor_tensor(
                out=m2, in0=mv[0][:, hs, 0], in1=mv[0][:, hs, 0], op=Alu.mult
            )
            nc.vector.tensor_scalar(
                out=mv[0][:, hs, 1], in0=sx_sq[:, hs], scalar1=1.0 / D,
                scalar2=None, op0=Alu.mult,
            )
            nc.vector.tensor_tensor(
                out=mv[0][:, hs, 1], in0=mv[0][:, hs, 1], in1=m2, op=Alu.subtract
            )
            stat_rows(0)
            natp.release()


            def proj_qk(w_sb, b_sb, nc_sb, src_T, ti, dst_ap, mt, qc):
                pm = ps_mm.tile([P, QW], f32, tag="mm")
                for kt in range(FT):
                    nc.tensor.matmul(
                        pm,
                        lhsT=w_sb[:, kt, mt * P : (mt + 1) * P],
                        rhs=src_T[:, kt, qc * QW : (qc + 1) * QW],
                        start=(kt == 0),
                        stop=False,
                    )
                # mean fix: rank-1 update (-colsum x mu)
                nc.tensor.matmul(
                    pm,
                    lhsT=nc_sb[0:1, mt * P : (mt + 1) * P],
                    rhs=mu_bf[ti][0:1, qc * QW : (qc + 1) * QW],
                    start=False,
                    stop=True,
                )
                cs = slice(qc * QW, (qc + 1) * QW)
                if with_bias:
                    t1 = work.tile([P, QW], f32, tag="projt")
                    nc.vector.tensor_tensor(out=t1, in0=pm, in1=rsb[ti][:, cs], op=Alu.mult)
                    nc.vector.tensor_scalar(
                        out=dst_ap,
                        in0=t1,
                        scalar1=b_sb[:, mt : mt + 1],
                        scalar2=None,
                        op0=Alu.add,
                    )
                else:
                    nc.vector.tensor_tensor(
                        out=dst_ap, in0=pm, in1=rsb[ti][:, cs], op=Alu.mult
                    )

            def emit_kT(mt):
                for qc2 in range(QC):
                    proj_qk(
                        wk_sb, bk_sb, nck_sb, cT_sb, 1,
                        kT[:, mt, qc2 * QW : (qc2 + 1) * QW], mt, qc2,
                    )

            def emit_v(st_lo, st_hi):
                for st in range(st_lo, st_hi):
                    pm = ps_av.tile([P, GI], f32, tag="av", name="pmv")
                    for kt in range(FT):
                        nc.tensor.matmul(
                            pm,
                            lhsT=cT_sb[:, kt, st * P : (st + 1) * P],
                            rhs=wv_sb[:, kt, :],
                            start=(kt == 0),
                            stop=False,
                        )
                    nc.tensor.matmul(
                        pm,
                        lhsT=mu_bf[1][0:1, st * P : (st + 1) * P],
                        rhs=ncv_sb[0:1, :],
                        start=False,
                        stop=True,
                    )
                    if with_bias:
                        t1 = work.tile([P, GI], f32, tag="vt")
                        nc.vector.tensor_scalar(
                            out=t1,
                            in0=pm,
                            scalar1=rs_nat[1][:, st : st + 1],
                            scalar2=None,
                            op0=Alu.mult,
                        )
                        nc.vector.tensor_tensor(
                            out=vext[:, st, :, 0:DH],
                            in0=t1.rearrange("p (h d) -> p h d", h=GH),
                            in1=bv_sb.rearrange("p (h d) -> p h d", h=GH),
                            op=Alu.add,
                        )
                    else:
                        nc.vector.tensor_scalar(
                            out=vext[:, st, :, 0:DH],
                            in0=pm.rearrange("p (h d) -> p h d", h=GH),
                            scalar1=rs_nat[1][:, st : st + 1],
                            scalar2=None,
                            op0=Alu.mult,
                        )

            expp = tc.alloc_tile_pool(name="expp", bufs=3)

            # ---- attention, software-pipelined at (qc, mt, par) units ----
            # PE order per unit j: [qT proj if new block] [sims U_j]
            # [avs U_{j-1}] [transposes/uT/o-proj when a block completes],
            # so PE's av work overlaps ScalarE's exp work on the next unit.
            un2s = {}  # (qs) staging for the current block

            def sims_unit(qc, mt, par):
                exs = [None, None]
                po = par * DH
                qt = qts[(mt, qc)]
                for kt2 in range(KT // 2):
                    half = kt2 // 4
                    if kt2 % 4 == 0:
                        exs[half] = expp.tile(
                            [P, KT // 2, QW],
                            bf16,
                            tag=f"exp{par}",
                            name=f"exp{par}h{half}",
                        )
                    pm = ps_sim.tile([P, 2, QW], f32, tag="sim")
                    for j in range(2):
                        kt = kt2 * 2 + j
                        nc.tensor.matmul(
                            pm[:, j, :],
                            lhsT=kT[po : po + DH, mt, kt * P : (kt + 1) * P],
                            rhs=qt[po : po + DH, :],
                            start=True,
                            stop=True,
                        )
                    nc.scalar.activation(
                        out=exs[half][:, (kt2 % 4) * 2 : (kt2 % 4) * 2 + 2, :],
                        in_=pm,
                        func=Act.Exp,
                        scale=SCALE,
                    )
                return exs

            def avs_unit(qc, mt, par, exs):
                h = 2 * mt + par
                for qs in range(4):
                    if par == 0:
                        un2s[qs] = unat.tile(
                            [P, 2, DH], f32, tag="un", name=f"un{qs}"
                        )
                    pu = ps_av.tile([P, DH + 1], f32, tag="av")
                    for kt in range(KT):
                        nc.tensor.matmul(
                            pu,
                            lhsT=exs[kt // 8][:, kt % 8, qs * P : (qs + 1) * P],
                            rhs=vext[:, kt, h, :],
                            start=(kt == 0),
                            stop=(kt == KT - 1),
                        )
                    rden = small.tile([P, 1], f32, tag="rden")
                    nc.vector.reciprocal(out=rden, in_=pu[:, DH : DH + 1])
                    nc.vector.tensor_scalar(
                        out=un2s[qs][:, par, :],
                        in0=pu[:, 0:DH],
                        scalar1=rden,
                        scalar2=None,
                        op0=Alu.mult,
                    )

            def finish_block(qc, mt):
                # transpose both heads' u into pair-stacked uT
                ut_ps = ps_mm.tile([P, QW], f32, tag="mm")
                for qs in range(4):
                    nc.tensor.transpose(
                        ut_ps[:, qs * P : (qs + 1) * P], un2s[qs][:, :, :],
                        ident_f32[:],
                    )
                nc.vector.tensor_copy(
                    out=uTp[mt][:, qc * QW : (qc + 1) * QW], in_=ut_ps
                )
                if mt != MT - 1:
                    return
                # output projection for this q-chunk
                for st in range(qc * QC, (qc + 1) * QC):
                    for nck2 in range(2):
                        pm = ps_mm.tile([P, QW], f32, tag="mm")
                        for mt2 in range(MT):
                            nc.tensor.matmul(
                                pm,
                                lhsT=uTp[mt2][:, st * P : (st + 1) * P],
                                rhs=wo_sb[:, mt2, nck2 * QW : (nck2 + 1) * QW],
                                start=(mt2 == 0),
                                stop=(mt2 == MT - 1),
                            )
                        o_sb = outp.tile([P, QW], f32, tag="o")
                        nc.vector.tensor_copy(out=o_sb, in_=pm)
                        nc.sync.dma_start(
                            o[st * P : (st + 1) * P, nck2 * QW : (nck2 + 1) * QW], o_sb
                        )

            # mt-major unit order: kT(mt1) is not needed until unit 8, and
            # the v projection hides under the first two units' exps
            units = [
                (qc, mt, par)
                for mt in range(MT)
                for qc in range(QC)
                for par in range(2)
            ]
            qts = {}
            exs_store = {}

            def do_sims(j):
                qc, mt, par = units[j]
                if par == 0:
                    qt = qtp.tile([P, QW], bf16, tag="qt", name=f"qt{mt}_{qc}")
                    qts[(mt, qc)] = qt
                    proj_qk(wq_sb, bq_sb, ncq_sb, xT_sb, 0, qt[:, :], mt, qc)
                exs_store[j] = sims_unit(qc, mt, par)

            def do_avs(j):
                qc, mt, par = units[j]
                avs_unit(qc, mt, par, exs_store.pop(j))
                if par == 1:
                    finish_block(qc, mt)

            emit_kT(0)
            do_sims(0)
            do_sims(1)
            emit_v(0, ST)
            for j in range(2, 16):
                do_avs(j - 2)
                if j == 8:
                    emit_kT(1)
                do_sims(j)
            do_avs(14)
            do_avs(15)

            expp.release()
            ct_pool.release()

    nc.finalize()
    return nc


def _prep_inputs(x, context, g1, b1, g2, b2, Wq, Wkv, Wout):
    """Fold LN affine into weights; build per-core input maps."""
    f32 = np.float32
    Wqf = (g1[:, None] * Wq).astype(f32)
    bqf = (b1 @ Wq).astype(f32)
    Wkvf = (g2[:, None] * Wkv).astype(f32)
    bkvf = (b2 @ Wkv).astype(f32)
    in_maps = []
    for c in range(8):
        b, g = c // 2, c % 2
        sl = slice(g * GI, (g + 1) * GI)
        slv = slice(INNER + g * GI, INNER + (g + 1) * GI)
        bq_g = bqf[sl.start : sl.stop]
        bk_g = bkvf[sl.start : sl.stop]
        bv_g = bkvf[slv.start : slv.stop]
        ncq_h = -Wqf[:, sl].sum(0)[None, :]
        nck_h = -Wkvf[:, sl].sum(0)[None, :]
        ncv_h = -Wkvf[:, slv].sum(0)[None, :]
        in_maps.append(
            {
                "xt": np.ascontiguousarray(x[b].astype(BF16).T),
                "ct": np.ascontiguousarray(context[b].astype(BF16).T),
                "xn": np.ascontiguousarray(x[b]).astype(BF16),
                "cn": np.ascontiguousarray(context[b]).astype(BF16),
                "ncq": np.ascontiguousarray(ncq_h).astype(BF16),
                "nck": np.ascontiguousarray(nck_h).astype(BF16),
                "ncv": np.ascontiguousarray(ncv_h).astype(BF16),
                "wq": np.ascontiguousarray(Wqf[:, sl]).astype(BF16),
                "wk": np.ascontiguousarray(Wkvf[:, sl]).astype(BF16),
                "wv": np.ascontiguousarray(Wkvf[:, slv]).astype(BF16),
                "wo": np.ascontiguousarray(Wout[sl]).astype(BF16),
                "bq": np.ascontiguousarray(bq_g.reshape(MT, P).T).astype(f32),
                "bk": np.ascontiguousarray(bk_g.reshape(MT, P).T).astype(f32),
                "bv": np.ascontiguousarray(np.broadcast_to(bv_g, (P, GI))).astype(f32),
            }
        )
    return in_maps


def kernel(x, context, g1, b1, g2, b2, Wq, Wkv, Wout, bout, _trace=False):
    from concourse.bass_utils import run_bass_kernel_spmd

    with_bias = bool(np.any(np.asarray(b1)) or np.any(np.asarray(b2)))
    key = ("nc", with_bias)
    if key not in _CACHE:
        _CACHE[key] = _build_nc(with_bias=with_bias)
    nc = _CACHE[key]

    in_maps = _prep_inputs(
        np.asarray(x, np.float32),
        np.asarray(context, np.float32),
        np.asarray(g1, np.float32),
        np.asarray(b1, np.float32),
        np.asarray(g2, np.float32),
        np.asarray(b2, np.float32),
        np.asarray(Wq, np.float32),
        np.asarray(Wkv, np.float32),
        np.asarray(Wout, np.float32),
    )
    res = run_bass_kernel_spmd(nc, in_maps, core_ids=list(range(8)), trace=_trace)
    out = np.empty((B, NSEQ, D), np.float32)
    for b in range(B):
        out[b] = res.results[2 * b]["o"] + res.results[2 * b + 1]["o"]
    out += np.asarray(bout, np.float32)
    _CACHE["last_result"] = res
    return out


# revision 47
# speedup vs baseline: 1.2344x; 1.0283x over previous
"""CrossAttention Trainium2 kernel (8 NeuronCores), v2.

Sharding: 8 cores = 4 batches x 2 head-groups (4 heads of 64 dims each).
Core c handles batch c//2 and inner-dim slice [g*256:(g+1)*256], g = c%2.
Each core computes a partial output [2048, 1024]; the host sums the two
partials per batch and adds bout.

v2 structure (vs v1): LN stats come from bn_stats on natural-layout tiles
(DVE) instead of PE ones-matmuls + ScalarE squares; the attention AV matmul
is flipped (lhsT=exp[keys,128q], rhs=v[keys,65]) so its PE cost is charged
on N=65 instead of N=512 and the softmax denominator lands as a
per-partition scalar; u is rebuilt into pair-stacked uT tiles with cheap PE
transposes; PSUM->SBUF copies for u/o go to the otherwise-idle Pool engine.

Device pipeline per core:
  bn_stats/bn_aggr (DVE) on natural x/c tiles -> mean/rstd [128, 16]
  -> PE-transpose stats to row layout; rstd rows broadcast (Pool) to [128, n]
  -> kT = Wk.T @ cT (+block-diag rank-1 mean fix), scaled by rstd rows
  -> v  = cT.T @ Wv (+rank-1), scaled per-partition rstd; ones column
  -> per (q-chunk, head-pair): qT tile proj, simT = kT_h.T @ qT_h,
     Exp on ScalarE (scale=1/8), u[q,65] = exp_chunk.T-style flipped matmul,
     normalize by denominator column during PSUM->SBUF copy,
     PE-transpose into pair-stacked uT
  -> o = uT.T @ Wout per q-chunk -> fp32 partial out.
"""

import numpy as np
import ml_dtypes

BF16 = ml_dtypes.bfloat16

B = 4
NSEQ = 2048
D = 1024
HEADS = 8
DH = 64
INNER = HEADS * DH  # 512
GI = INNER // 2  # 256 inner dims per core (4 heads)
GH = 4  # heads per core
EPS = 1e-5
SCALE = DH ** -0.5

P = 128
ST = NSEQ // P  # 16 seq tiles
FT = D // P  # 8 feature tiles
MT = GI // P  # 2 inner tiles (head pairs)
QW = 512  # q chunk width
QC = NSEQ // QW  # 4 q chunks
KT = NSEQ // P  # 16 krow tiles

_CACHE = {}


def _build_nc(with_bias=True):
    import concourse.mybir as mybir
    import concourse.tile as tile
    from concourse import bacc, masks

    f32 = mybir.dt.float32
    bf16 = mybir.dt.bfloat16
    Alu = mybir.AluOpType
    Act = mybir.ActivationFunctionType

    nc = bacc.Bacc(None, target_bir_lowering=False)

    xt = nc.dram_tensor("xt", [D, NSEQ], bf16, kind="ExternalInput")
    ct = nc.dram_tensor("ct", [D, NSEQ], bf16, kind="ExternalInput")
    xn = nc.dram_tensor("xn", [NSEQ, D], bf16, kind="ExternalInput")
    cn = nc.dram_tensor("cn", [NSEQ, D], bf16, kind="ExternalInput")
    ncq = nc.dram_tensor("ncq", [1, GI], bf16, kind="ExternalInput")
    nck = nc.dram_tensor("nck", [1, GI], bf16, kind="ExternalInput")
    ncv = nc.dram_tensor("ncv", [1, GI], bf16, kind="ExternalInput")
    wq = nc.dram_tensor("wq", [D, GI], bf16, kind="ExternalInput")
    wk = nc.dram_tensor("wk", [D, GI], bf16, kind="ExternalInput")
    wv = nc.dram_tensor("wv", [D, GI], bf16, kind="ExternalInput")
    wo = nc.dram_tensor("wo", [GI, D], bf16, kind="ExternalInput")
    bq = nc.dram_tensor("bq", [P, MT], f32, kind="ExternalInput")
    bk = nc.dram_tensor("bk", [P, MT], f32, kind="ExternalInput")
    bv = nc.dram_tensor("bv", [P, GI], f32, kind="ExternalInput")
    o = nc.dram_tensor("o", [NSEQ, D], f32, kind="ExternalOutput")

    with tile.TileContext(nc) as tc:
        with (
            tc.tile_pool(name="const", bufs=1) as const,
            tc.tile_pool(name="persist", bufs=1) as persist,
            tc.tile_pool(name="work", bufs=2) as work,
            tc.tile_pool(name="qtp", bufs=3) as qtp,
            tc.tile_pool(name="stats", bufs=4) as stats,
            tc.tile_pool(name="statp", bufs=1) as statp,
            tc.tile_pool(name="small", bufs=4) as small,
            tc.tile_pool(name="unat", bufs=8) as unat,
            tc.tile_pool(name="outp", bufs=4) as outp,
            tc.tile_pool(name="ps_mm", bufs=2, space="PSUM") as ps_mm,
            tc.tile_pool(name="ps_sim", bufs=2, space="PSUM") as ps_sim,
            tc.tile_pool(name="ps_av", bufs=2, space="PSUM") as ps_av,
        ):
            # ---- constants / weights in SBUF ----
            # weight tiles; DMAs are issued later in consumption order
            wq_sb = const.tile([P, FT, GI], bf16)
            wk_sb = const.tile([P, FT, GI], bf16)
            wv_sb = const.tile([P, FT, GI], bf16)
            wo_sb = const.tile([P, MT, D], bf16)
            ncq_sb = const.tile([1, GI], bf16)
            nc.sync.dma_start(ncq_sb, ncq[:, :])
            nck_sb = const.tile([1, GI], bf16)
            nc.sync.dma_start(nck_sb, nck[:, :])
            ncv_sb = const.tile([1, GI], bf16)
            nc.sync.dma_start(ncv_sb, ncv[:, :])
            bq_sb = const.tile([P, MT], f32)
            nc.sync.dma_start(bq_sb, bq[:, :])
            bk_sb = const.tile([P, MT], f32)
            nc.sync.dma_start(bk_sb, bk[:, :])
            bv_sb = const.tile([P, GI], f32)
            nc.sync.dma_start(bv_sb, bv[:, :])
            eps_sb = const.tile([P, 1], f32)
            nc.vector.memset(eps_sb, EPS)
            ident_f32 = const.tile([P, P], f32)
            masks.make_identity(nc, ident_f32[:])

            # ---- persistent activations ----
            ct_pool = tc.alloc_tile_pool(name="ctp", bufs=1)
            cT_sb = ct_pool.tile([P, FT, NSEQ], bf16)
            natp = tc.alloc_tile_pool(name="natp", bufs=4)
            xT_sb = persist.tile([P, FT, NSEQ], bf16)
            kT = persist.tile([P, MT, NSEQ], bf16)
            vext = persist.tile([P, KT, GH, DH + 1], bf16)
            uTp = [
                persist.tile([P, NSEQ], bf16, name=f"uTp{m}", tag=f"uTp{m}")
                for m in range(MT)
            ]
            rsb = [
                persist.tile([P, NSEQ], bf16, name=f"rsb{i}", tag=f"rsb{i}")
                for i in range(2)
            ]
            # stats accumulators: mv[ti][:, st, 0]=mean, [..,1]=var
            mv = [
                statp.tile([P, ST, 2], f32, name=f"mv{i}", tag=f"mv{i}")
                for i in range(2)
            ]
            rs_nat = [
                statp.tile([P, ST], f32, name=f"rsn{i}", tag=f"rsn{i}")
                for i in range(2)
            ]
            # row-layout stats at partition 0 (for rank-1 mean fixes / pb)
            mu_bf = [
                statp.tile([1, NSEQ], bf16, name=f"mubf{i}", tag=f"mubf{i}")
                for i in range(2)
            ]
            rs_row = [
                statp.tile([1, NSEQ], bf16, name=f"rsrow{i}", tag=f"rsrow{i}")
                for i in range(2)
            ]

            nc.vector.memset(vext[:, :, :, DH], 1.0)

            # x sums for the ScalarE-computed half of x stats
            sx_sum = statp.tile([P, ST], f32, name="sxsum", tag="sxsum")
            sx_sq = statp.tile([P, ST], f32, name="sxsq", tag="sxsq")

            # ---- Phase 1: stats + loads; DMAs batched and issued in
            # consumption order (c-nat, cT, wk -> x-nat, xT blocks, wq, wv) --
            xt_r = xt.rearrange("(ko p) s -> p ko s", p=P)
            ct_r = ct.rearrange("(ko p) s -> p ko s", p=P)
            wq_r = wq.rearrange("(ko p) m -> p ko m", p=P)
            wk_r = wk.rearrange("(ko p) m -> p ko m", p=P)
            wv_r = wv.rearrange("(ko p) m -> p ko m", p=P)
            wo_r = wo.rearrange("(mt p) d -> p mt d", p=P)

            BT = 4  # seq tiles per big natural tile
            nat_tiles = {}

            def nat_load(src, ti, bt):
                t = natp.tile([P, BT, D], bf16, tag="nat", name=f"nat{ti}_{bt}")
                nc.sync.dma_start(
                    t, src[bt * BT * P : (bt + 1) * BT * P, :].rearrange(
                        "(s p) d -> p s d", p=P
                    )
                )
                nat_tiles[(ti, bt)] = t

            def dve_stats(ti, bt, s):
                t = nat_tiles[(ti, bt)]
                st = bt * BT + s
                bst = stats.tile([P, 2, 6], f32, tag="bst")
                for c2 in range(2):
                    nc.vector.bn_stats(
                        out=bst[:, c2, :], in_=t[:, s, c2 * 512 : (c2 + 1) * 512]
                    )
                nc.vector.bn_aggr(out=mv[ti][:, st, :], in_=bst)

            def se_stats(ti, bt, s):
                # x stats on ScalarE: Square+accum and Copy+accum
                t = nat_tiles[(ti, bt)]
                st = bt * BT + s
                scr = stats.tile([P, D], bf16, tag="sescr")
                nc.scalar.activation(
                    out=scr, in_=t[:, s, :], func=Act.Square,
                    accum_out=sx_sq[:, st : st + 1],
                )
                scr2 = stats.tile([P, D], bf16, tag="sescr")
                nc.scalar.activation(
                    out=scr2, in_=t[:, s, :], func=Act.Copy,
                    accum_out=sx_sum[:, st : st + 1],
                )

            def stat_rows(ti):
                # rstd per seq position (natural layout)
                sd = stats.tile([P, ST], f32, tag="sd")
                nc.scalar.activation(
                    out=sd, in_=mv[ti][:, :, 1], func=Act.Sqrt, bias=eps_sb
                )
                nc.vector.reciprocal(out=rs_nat[ti], in_=sd)
                # transpose mean and rstd to row layout via PE
                pmu = ps_mm.tile([P, QW], f32, tag="mm")
                nc.tensor.transpose(pmu[0:ST, 0:P], mv[ti][:, :, 0], ident_f32[:])
                nc.tensor.transpose(pmu[0:ST, P : 2 * P], rs_nat[ti][:, :], ident_f32[:])
                rsT = stats.tile([ST, P], bf16, tag="rsT")
                nc.vector.tensor_copy(out=rsT, in_=pmu[0:ST, P : 2 * P])
                muT = stats.tile([ST, P], bf16, tag="muT")
                nc.vector.tensor_copy(out=muT, in_=pmu[0:ST, 0:P])
                nc.sync.dma_start(mu_bf[ti][0:1, :], muT[:, :])
                nc.sync.dma_start(rs_row[ti][0:1, :], rsT[:, :])
                nc.gpsimd.partition_broadcast(rsb[ti], rs_row[ti])


            # DMA dispatch order tuned to arrival deadlines:
            # c-nat (stats) -> wk -> x-nat 0/1 (ScalarE stats start early)
            # -> cT blocks (kT rhs) -> x-nat 2/3 -> xT block 0 / wq (first
            # q-chunk) -> wv -> remaining xT -> wo
            def dma_cT(qc2):
                nc.sync.dma_start(
                    cT_sb[:, :, qc2 * QW : (qc2 + 1) * QW],
                    ct_r[:, :, qc2 * QW : (qc2 + 1) * QW],
                )

            def dma_xT(qc2):
                nc.sync.dma_start(
                    xT_sb[:, :, qc2 * QW : (qc2 + 1) * QW],
                    xt_r[:, :, qc2 * QW : (qc2 + 1) * QW],
                )

            for bt in range(ST // BT):
                nat_load(cn, 1, bt)
            nc.sync.dma_start(wk_sb, wk_r)
            # c stats for bt 0,1 then x loads take their ring slots
            for s in range(BT):
                dve_stats(1, 0, s)
            nat_load(xn, 0, 0)
            for s in range(BT):
                se_stats(0, 0, s)
            for s in range(BT):
                dve_stats(1, 1, s)
            nat_load(xn, 0, 1)
            for s in range(BT):
                se_stats(0, 1, s)
            dma_cT(0)
            dma_cT(1)
            dma_xT(0)
            nc.sync.dma_start(wq_sb, wq_r)
            for s in range(BT):
                dve_stats(1, 2, s)
            nat_load(xn, 0, 2)
            for s in range(BT):
                dve_stats(1, 3, s)
            stat_rows(1)
            nat_load(xn, 0, 3)
            dma_cT(2)
            dma_cT(3)
            nc.sync.dma_start(wv_sb, wv_r)
            for qc2 in range(1, QC):
                dma_xT(qc2)
            nc.sync.dma_start(wo_sb, wo_r)
            # x stats for bt 2,3 on DVE
            for bt in range(2, ST // BT):
                for s in range(BT):
                    dve_stats(0, bt, s)
            # finalize ScalarE-half x stats: mean=sum/D, var=sumsq/D-mean^2
            hs = slice(0, ST // 2)
            nc.vector.tensor_scalar(
                out=mv[0][:, hs, 0], in0=sx_sum[:, hs], scalar1=1.0 / D,
                scalar2=None, op0=Alu.mult,
            )
            m2 = stats.tile([P, ST // 2], f32, tag="m2")
            nc.vector.tensor_tensor(
                out=m2, in0=mv[0][:, hs, 0], in1=mv[0][:, hs, 0], op=Alu.mult
            )
            nc.vector.tensor_scalar(
                out=mv[0][:, hs, 1], in0=sx_sq[:, hs], scalar1=1.0 / D,
                scalar2=None, op0=Alu.mult,
            )
            nc.vector.tensor_tensor(
                out=mv[0][:, hs, 1], in0=mv[0][:, hs, 1], in1=m2, op=Alu.subtract
            )
            natp.release()


            def proj_qk(w_sb, b_sb, nc_sb, src_T, ti, dst_ap, mt, qc):
                pm = ps_mm.tile([P, QW], f32, tag="mm")
                for kt in range(FT):
                    nc.tensor.matmul(
                        pm,
                        lhsT=w_sb[:, kt, mt * P : (mt + 1) * P],
                        rhs=src_T[:, kt, qc * QW : (qc + 1) * QW],
                        start=(kt == 0),
                        stop=False,
                    )
                # mean fix: rank-1 update (-colsum x mu)
                nc.tensor.matmul(
                    pm,
                    lhsT=nc_sb[0:1, mt * P : (mt + 1) * P],
                    rhs=mu_bf[ti][0:1, qc * QW : (qc + 1) * QW],
                    start=False,
                    stop=True,
                )
                cs = slice(qc * QW, (qc + 1) * QW)
                if with_bias:
                    t1 = work.tile([P, QW], f32, tag="projt")
                    nc.vector.tensor_tensor(out=t1, in0=pm, in1=rsb[ti][:, cs], op=Alu.mult)
                    nc.vector.tensor_scalar(
                        out=dst_ap,
                        in0=t1,
                        scalar1=b_sb[:, mt : mt + 1],
                        scalar2=None,
                        op0=Alu.add,
                    )
                else:
                    nc.vector.tensor_tensor(
                        out=dst_ap, in0=pm, in1=rsb[ti][:, cs], op=Alu.mult
                    )

            def emit_kT(mt):
                for qc2 in range(QC):
                    proj_qk(
                        wk_sb, bk_sb, nck_sb, cT_sb, 1,
                        kT[:, mt, qc2 * QW : (qc2 + 1) * QW], mt, qc2,
                    )

            def emit_v(st_lo, st_hi):
                for st in range(st_lo, st_hi):
                    pm = ps_av.tile([P, GI], f32, tag="av", name="pmv")
                    for kt in range(FT):
                        nc.tensor.matmul(
                            pm,
                            lhsT=cT_sb[:, kt, st * P : (st + 1) * P],
                            rhs=wv_sb[:, kt, :],
                            start=(kt == 0),
                            stop=False,
                        )
                    nc.tensor.matmul(
                        pm,
                        lhsT=mu_bf[1][0:1, st * P : (st + 1) * P],
                        rhs=ncv_sb[0:1, :],
                        start=False,
                        stop=True,
                    )
                    if with_bias:
                        t1 = work.tile([P, GI], f32, tag="vt")
                        nc.vector.tensor_scalar(
                            out=t1,
                            in0=pm,
                            scalar1=rs_nat[1][:, st : st + 1],
                            scalar2=None,
                            op0=Alu.mult,
                        )
                        nc.vector.tensor_tensor(
                            out=vext[:, st, :, 0:DH],
                            in0=t1.rearrange("p (h d) -> p h d", h=GH),
                            in1=bv_sb.rearrange("p (h d) -> p h d", h=GH),
                            op=Alu.add,
                        )
                    else:
                        nc.vector.tensor_scalar(
                            out=vext[:, st, :, 0:DH],
                            in0=pm.rearrange("p (h d) -> p h d", h=GH),
                            scalar1=rs_nat[1][:, st : st + 1],
                            scalar2=None,
                            op0=Alu.mult,
                        )

            expp = tc.alloc_tile_pool(name="expp", bufs=3)

            # ---- attention, software-pipelined at (qc, mt, par) units ----
            # PE order per unit j: [qT proj if new block] [sims U_j]
            # [avs U_{j-1}] [transposes/uT/o-proj when a block completes],
            # so PE's av work overlaps ScalarE's exp work on the next unit.
            un2s = {}  # (qs) staging for the current block

            def sims_unit(qc, mt, par):
                exs = [None, None]
                po = par * DH
                qt = qts[(mt, qc)]
                for kt2 in range(KT // 2):
                    half = kt2 // 4
                    if kt2 % 4 == 0:
                        exs[half] = expp.tile(
                            [P, KT // 2, QW],
                            bf16,
                            tag=f"exp{par}",
                            name=f"exp{par}h{half}",
                        )
                    pm = ps_sim.tile([P, 2, QW], f32, tag="sim")
                    for j in range(2):
                        kt = kt2 * 2 + j
                        nc.tensor.matmul(
                            pm[:, j, :],
                            lhsT=kT[po : po + DH, mt, kt * P : (kt + 1) * P],
                            rhs=qt[po : po + DH, :],
                            start=True,
                            stop=True,
                        )
                    nc.scalar.activation(
                        out=exs[half][:, (kt2 % 4) * 2 : (kt2 % 4) * 2 + 2, :],
                        in_=pm,
                        func=Act.Exp,
                        scale=SCALE,
                    )
                return exs

            def avs_unit(qc, mt, par, exs):
                h = 2 * mt + par
                for qs in range(4):
                    if par == 0:
                        un2s[qs] = unat.tile(
                            [P, 2, DH], f32, tag="un", name=f"un{qs}"
                        )
                    pu = ps_av.tile([P, DH + 1], f32, tag="av")
                    for kt in range(KT):
                        nc.tensor.matmul(
                            pu,
                            lhsT=exs[kt // 8][:, kt % 8, qs * P : (qs + 1) * P],
                            rhs=vext[:, kt, h, :],
                            start=(kt == 0),
                            stop=(kt == KT - 1),
                        )
                    rden = small.tile([P, 1], f32, tag="rden")
                    nc.vector.reciprocal(out=rden, in_=pu[:, DH : DH + 1])
                    nc.vector.tensor_scalar(
                        out=un2s[qs][:, par, :],
                        in0=pu[:, 0:DH],
                        scalar1=rden,
                        scalar2=None,
                        op0=Alu.mult,
                    )

            def finish_block(qc, mt):
                # transpose both heads' u into pair-stacked uT
                ut_ps = ps_mm.tile([P, QW], f32, tag="mm")
                for qs in range(4):
                    nc.tensor.transpose(
                        ut_ps[:, qs * P : (qs + 1) * P], un2s[qs][:, :, :],
                        ident_f32[:],
                    )
                nc.vector.tensor_copy(
                    out=uTp[mt][:, qc * QW : (qc + 1) * QW], in_=ut_ps
                )
                if mt != MT - 1:
                    return
                # output projection for this q-chunk
                for st in range(qc * QC, (qc + 1) * QC):
                    for nck2 in range(2):
                        pm = ps_mm.tile([P, QW], f32, tag="mm")
                        for mt2 in range(MT):
                            nc.tensor.matmul(
                                pm,
                                lhsT=uTp[mt2][:, st * P : (st + 1) * P],
                                rhs=wo_sb[:, mt2, nck2 * QW : (nck2 + 1) * QW],
                                start=(mt2 == 0),
                                stop=(mt2 == MT - 1),
                            )
                        o_sb = outp.tile([P, QW], f32, tag="o")
                        nc.vector.tensor_copy(out=o_sb, in_=pm)
                        nc.sync.dma_start(
                            o[st * P : (st + 1) * P, nck2 * QW : (nck2 + 1) * QW], o_sb
                        )

            # mt-major unit order: kT(mt1) is not needed until unit 8, and
            # the v projection hides under the first two units' exps
            units = [
                (qc, mt, par)
                for mt in range(MT)
                for qc in range(QC)
                for par in range(2)
            ]
            qts = {}
            exs_store = {}

            def do_sims(j):
                qc, mt, par = units[j]
                if par == 0:
                    qt = qtp.tile([P, QW], bf16, tag="qt", name=f"qt{mt}_{qc}")
                    qts[(mt, qc)] = qt
                    proj_qk(wq_sb, bq_sb, ncq_sb, xT_sb, 0, qt[:, :], mt, qc)
                exs_store[j] = sims_unit(qc, mt, par)

            def do_avs(j):
                qc, mt, par = units[j]
                avs_unit(qc, mt, par, exs_store.pop(j))
                if par == 1:
                    finish_block(qc, mt)

            emit_kT(0)
            stat_rows(0)
            do_sims(0)
            do_sims(1)
            emit_v(0, ST)
            for j in range(2, 16):
                do_avs(j - 2)
                if 4 <= j < 8:
                    # one kT(mt1) tile per iteration, hidden under exps
                    proj_qk(
                        wk_sb, bk_sb, nck_sb, cT_sb, 1,
                        kT[:, 1, (j - 4) * QW : (j - 3) * QW], 1, j - 4,
                    )
                do_sims(j)
            do_avs(14)
            do_avs(15)

            expp.release()
            ct_pool.release()

    nc.finalize()
    return nc


def _prep_inputs(x, context, g1, b1, g2, b2, Wq, Wkv, Wout):
    """Fold LN affine into weights; build per-core input maps."""
    f32 = np.float32
    Wqf = (g1[:, None] * Wq).astype(f32)
    bqf = (b1 @ Wq).astype(f32)
    Wkvf = (g2[:, None] * Wkv).astype(f32)
    bkvf = (b2 @ Wkv).astype(f32)
    in_maps = []
    for c in range(8):
        b, g = c // 2, c % 2
        sl = slice(g * GI, (g + 1) * GI)
        slv = slice(INNER + g * GI, INNER + (g + 1) * GI)
        bq_g = bqf[sl.start : sl.stop]
        bk_g = bkvf[sl.start : sl.stop]
        bv_g = bkvf[slv.start : slv.stop]
        ncq_h = -Wqf[:, sl].sum(0)[None, :]
        nck_h = -Wkvf[:, sl].sum(0)[None, :]
        ncv_h = -Wkvf[:, slv].sum(0)[None, :]
        in_maps.append(
            {
                "xt": np.ascontiguousarray(x[b].astype(BF16).T),
                "ct": np.ascontiguousarray(context[b].astype(BF16).T),
                "xn": np.ascontiguousarray(x[b]).astype(BF16),
                "cn": np.ascontiguousarray(context[b]).astype(BF16),
                "ncq": np.ascontiguousarray(ncq_h).astype(BF16),
                "nck": np.ascontiguousarray(nck_h).astype(BF16),
                "ncv": np.ascontiguousarray(ncv_h).astype(BF16),
                "wq": np.ascontiguousarray(Wqf[:, sl]).astype(BF16),
                "wk": np.ascontiguousarray(Wkvf[:, sl]).astype(BF16),
                "wv": np.ascontiguousarray(Wkvf[:, slv]).astype(BF16),
                "wo": np.ascontiguousarray(Wout[sl]).astype(BF16),
                "bq": np.ascontiguousarray(bq_g.reshape(MT, P).T).astype(f32),
                "bk": np.ascontiguousarray(bk_g.reshape(MT, P).T).astype(f32),
                "bv": np.ascontiguousarray(np.broadcast_to(bv_g, (P, GI))).astype(f32),
            }
        )
    return in_maps


def kernel(x, context, g1, b1, g2, b2, Wq, Wkv, Wout, bout, _trace=False):
    from concourse.bass_utils import run_bass_kernel_spmd

    with_bias = bool(np.any(np.asarray(b1)) or np.any(np.asarray(b2)))
    key = ("nc", with_bias)
    if key not in _CACHE:
        _CACHE[key] = _build_nc(with_bias=with_bias)
    nc = _CACHE[key]

    in_maps = _prep_inputs(
        np.asarray(x, np.float32),
        np.asarray(context, np.float32),
        np.asarray(g1, np.float32),
        np.asarray(b1, np.float32),
        np.asarray(g2, np.float32),
        np.asarray(b2, np.float32),
        np.asarray(Wq, np.float32),
        np.asarray(Wkv, np.float32),
        np.asarray(Wout, np.float32),
    )
    res = run_bass_kernel_spmd(nc, in_maps, core_ids=list(range(8)), trace=_trace)
    out = np.empty((B, NSEQ, D), np.float32)
    for b in range(B):
        out[b] = res.results[2 * b]["o"] + res.results[2 * b + 1]["o"]
    out += np.asarray(bout, np.float32)
    _CACHE["last_result"] = res
    return out
